# revision 2
# baseline (speedup 1.0000x reference)
"""BinaryLinear forward on 8 Trainium2 NeuronCores.

Computes out = x @ sign(weight).T for x:[16384,2048] (values in {-1,+1}),
weight:[2048,2048], out:[16384,2048] fp32.

Strategy (data-parallel, per sharding hint):
 - shard x rows across the 8 cores (2048 rows each), replicate the weight.
 - binarized operands are exactly representable in bf16, and the fp32 PSUM
   accumulation of +/-1 products over K=2048 is exact (integers <= 2048),
   so the device matmul runs in bf16 with bit-exact fp32 results.
 - per core: C[2048,2048] = xT_shard.T @ wT with both operands pre-transposed
   on the host so K lands on the SBUF partition dim with unit-stride DMAs.
   Both operands stay SBUF-resident (8.4 MB each); compute is structured in
   (o-block, m-half) phases with the K loop outermost inside a phase so the
   initial HBM load overlaps the first phase's matmuls.
"""

import numpy as np
import ml_dtypes

import concourse.mybir as mybir
import concourse.tile as tile
from concourse import bacc
from concourse.bass_utils import run_bass_kernel_spmd

M, K, O = 16384, 2048, 2048
N_CORES = 8
MS = M // N_CORES          # 2048 rows of x per core
P = 128
KO = K // P                # 16 k-subtiles
NB = 512                   # matmul moving free dim (one PSUM bank of fp32)
NJ = O // NB               # 4 o-blocks
MO = MS // P               # 16 m-blocks per core
MH = 8                     # m-blocks per phase (8 PSUM banks)

_CACHE = {}


def _build():
    if "nc" in _CACHE:
        return _CACHE["nc"]

    nc = bacc.Bacc("TRN2", target_bir_lowering=False, debug=False,
                   num_devices=N_CORES)
    xT = nc.dram_tensor("xT", [K, MS], mybir.dt.bfloat16, kind="ExternalInput")
    wT = nc.dram_tensor("wT", [K, O], mybir.dt.bfloat16, kind="ExternalInput")
    out = nc.dram_tensor("out", [MS, O], mybir.dt.float32,
                         kind="ExternalOutput")

    xT_v = xT.ap().rearrange("(ko pi) m -> pi ko m", pi=P)
    wT_v = wT.ap().rearrange("(ko pi) o -> pi ko o", pi=P)
    out_v = out.ap().rearrange("(mo pi) o -> pi mo o", pi=P)

    with tile.TileContext(nc) as tc:
        with tc.tile_pool(name="xres", bufs=1) as x_pool, \
             tc.tile_pool(name="wres", bufs=1) as w_pool, \
             tc.tile_pool(name="outs", bufs=8) as out_pool, \
             tc.tile_pool(name="psum", bufs=8, space="PSUM") as psum_pool:

            # Resident input tiles. Separate tiles per (ko) / (ko, j) chunk so
            # the Tile scheduler can start matmuls as soon as each chunk lands.
            x_t = []
            w_t = [[None] * NJ for _ in range(KO)]
            for ko in range(KO):
                xt = x_pool.tile([P, MS], mybir.dt.bfloat16, tag=f"x{ko}")
                nc.sync.dma_start(xt[:], xT_v[:, ko])
                x_t.append(xt)
                wt = w_pool.tile([P, NB], mybir.dt.bfloat16, tag=f"w{ko}_0")
                nc.sync.dma_start(wt[:], wT_v[:, ko, 0:NB])
                w_t[ko][0] = wt
            for j in range(1, NJ):
                for ko in range(KO):
                    wt = w_pool.tile([P, NB], mybir.dt.bfloat16,
                                     tag=f"w{ko}_{j}")
                    nc.sync.dma_start(wt[:], wT_v[:, ko, j * NB:(j + 1) * NB])
                    w_t[ko][j] = wt

            # Compute phases: (o-block j, m-half). K outermost within a phase
            # so the first phase pipelines against the input DMA stream.
            for j in range(NJ):
                for mh in range(MO // MH):
                    psums = [psum_pool.tile([P, NB], mybir.dt.float32,
                                            tag="ps", name=f"ps_{j}_{mh}_{i}")
                             for i in range(MH)]
                    for ko in range(KO):
                        for mi in range(MH):
                            mo = mh * MH + mi
                            nc.tensor.matmul(
                                psums[mi][:],
                                x_t[ko][:, mo * P:(mo + 1) * P],
                                w_t[ko][j][:],
                                start=(ko == 0),
                                stop=(ko == KO - 1),
                            )
                    for mi in range(MH):
                        mo = mh * MH + mi
                        ot = out_pool.tile([P, NB], mybir.dt.float32, tag="ot")
                        nc.vector.tensor_copy(out=ot[:], in_=psums[mi][:])
                        nc.sync.dma_start(
                            out_v[:, mo, j * NB:(j + 1) * NB], ot[:])

    nc.compile()
    _CACHE["nc"] = nc
    return nc


def kernel(x, weight):
    nc = _build()

    # sign(sign(w) + 0.5): maps 0 -> +1, else +/-1  (exact in bf16)
    bw = np.sign(np.sign(weight, dtype=np.float32) + np.float32(0.5))
    wT_h = np.ascontiguousarray(bw.T.astype(ml_dtypes.bfloat16))   # [K, O]
    xT_h = np.ascontiguousarray(x.T.astype(ml_dtypes.bfloat16))    # [K, M]

    in_maps = [
        {"xT": np.ascontiguousarray(xT_h[:, c * MS:(c + 1) * MS]), "wT": wT_h}
        for c in range(N_CORES)
    ]
    res = run_bass_kernel_spmd(nc, in_maps, core_ids=list(range(N_CORES)))
    return np.concatenate([res.results[c]["out"] for c in range(N_CORES)],
                          axis=0)


# revision 3
# speedup vs baseline: 1.9137x; 1.9137x over previous
"""BinaryLinear forward on 8 Trainium2 NeuronCores.

Computes out = x @ sign(weight).T for x:[16384,2048] (values in {-1,+1}),
weight:[2048,2048], out:[16384,2048] fp32.

Strategy (data-parallel, per sharding hint):
 - shard x rows across the 8 cores (2048 rows each), replicate the weight.
 - binarized operands are exactly representable in bf16, and the fp32 PSUM
   accumulation of +/-1 products over K=2048 is exact (integers <= 2048),
   so the device matmul runs in bf16 with bit-exact fp32 results.
 - per core: C[2048,2048] = xT_shard.T @ wT with both operands pre-transposed
   on the host so K lands on the SBUF partition dim with unit-stride DMAs.
   Both operands stay SBUF-resident (8.4 MB each); compute is structured in
   (o-block, m-half) phases with the K loop outermost inside a phase so the
   initial HBM load overlaps the first phase's matmuls.
"""

import numpy as np
import ml_dtypes

import concourse.mybir as mybir
import concourse.tile as tile
from concourse import bacc
from concourse.bass_utils import run_bass_kernel_spmd

M, K, O = 16384, 2048, 2048
N_CORES = 8
MS = M // N_CORES          # 2048 rows of x per core
P = 128
KO = K // P                # 16 k-subtiles
NB = 512                   # matmul moving free dim (one PSUM bank of fp32)
NJ = O // NB               # 4 o-blocks
MO = MS // P               # 16 m-blocks per core
MH = 8                     # m-blocks per phase (8 PSUM banks)

_CACHE = {}


def _build():
    if "nc" in _CACHE:
        return _CACHE["nc"]

    nc = bacc.Bacc("TRN2", target_bir_lowering=False, debug=False,
                   num_devices=N_CORES)
    xT = nc.dram_tensor("xT", [K, MS], mybir.dt.bfloat16, kind="ExternalInput")
    wT = nc.dram_tensor("wT", [K, O], mybir.dt.bfloat16, kind="ExternalInput")
    out = nc.dram_tensor("out", [MS, O], mybir.dt.float32,
                         kind="ExternalOutput")

    xT_v = xT.ap().rearrange("(ko pi) m -> pi ko m", pi=P)
    wT_v = wT.ap().rearrange("(ko pi) o -> pi ko o", pi=P)
    out_v = out.ap().rearrange("(mo pi) o -> pi mo o", pi=P)

    with tile.TileContext(nc) as tc:
        with tc.tile_pool(name="xres", bufs=1) as x_pool, \
             tc.tile_pool(name="wres", bufs=1) as w_pool, \
             tc.tile_pool(name="outs", bufs=8) as out_pool, \
             tc.tile_pool(name="psum", bufs=8, space="PSUM") as psum_pool:

            # Resident input tiles. Separate tiles per (ko) / (ko, j) chunk so
            # the Tile scheduler can start matmuls as soon as each chunk lands.
            x_t = []
            w_t = [[None] * NJ for _ in range(KO)]
            for ko in range(KO):
                xt = x_pool.tile([P, MS], mybir.dt.bfloat16, tag=f"x{ko}")
                nc.sync.dma_start(xt[:], xT_v[:, ko])
                x_t.append(xt)
                wt = w_pool.tile([P, NB], mybir.dt.bfloat16, tag=f"w{ko}_0")
                nc.sync.dma_start(wt[:], wT_v[:, ko, 0:NB])
                w_t[ko][0] = wt
            for j in range(1, NJ):
                for ko in range(KO):
                    wt = w_pool.tile([P, NB], mybir.dt.bfloat16,
                                     tag=f"w{ko}_{j}")
                    nc.sync.dma_start(wt[:], wT_v[:, ko, j * NB:(j + 1) * NB])
                    w_t[ko][j] = wt

            # Compute phases: (o-block j, m-half). K outermost within a phase
            # so the first phase pipelines against the input DMA stream.
            for j in range(NJ):
                for mh in range(MO // MH):
                    psums = [psum_pool.tile([P, NB], mybir.dt.float32,
                                            tag="ps", name=f"ps_{j}_{mh}_{i}")
                             for i in range(MH)]
                    for ko in range(KO):
                        for mi in range(MH):
                            mo = mh * MH + mi
                            nc.tensor.matmul(
                                psums[mi][:],
                                x_t[ko][:, mo * P:(mo + 1) * P],
                                w_t[ko][j][:],
                                start=(ko == 0),
                                stop=(ko == KO - 1),
                            )
                    for mi in range(MH):
                        mo = mh * MH + mi
                        ot = out_pool.tile([P, NB], mybir.dt.float32, tag="ot")
                        nc.vector.tensor_copy(out=ot[:], in_=psums[mi][:])
                        nc.sync.dma_start(
                            out_v[:, mo, j * NB:(j + 1) * NB], ot[:])

    nc.compile()
    _CACHE["nc"] = nc
    return nc


def prepare_in_maps(x, weight):
    # sign(sign(w) + 0.5): maps 0 -> +1, else +/-1  (exact in bf16)
    bw = np.sign(np.sign(weight, dtype=np.float32) + np.float32(0.5))
    wT_h = np.ascontiguousarray(bw.T.astype(ml_dtypes.bfloat16))   # [K, O]
    xT_h = np.ascontiguousarray(x.T.astype(ml_dtypes.bfloat16))    # [K, M]
    return [
        {"xT": np.ascontiguousarray(xT_h[:, c * MS:(c + 1) * MS]), "wT": wT_h}
        for c in range(N_CORES)
    ]


def gather_output(results):
    return np.concatenate([results[c]["out"] for c in range(N_CORES)], axis=0)


def kernel(x, weight):
    nc = _build()
    in_maps = prepare_in_maps(x, weight)
    res = run_bass_kernel_spmd(nc, in_maps, core_ids=list(range(N_CORES)))
    return np.concatenate([res.results[c]["out"] for c in range(N_CORES)],
                          axis=0)
